# revision 32
# baseline (speedup 1.0000x reference)
"""Trainium2 Bass kernel for the MIL fc/batch-ensemble/VQ model.

Strategy (8 NeuronCores, instance-parallel):
  - Host: transpose h -> hT [1024, 50000] (fp32), shard columns 8 ways.
    Pack Ua/Ub into W2 [512, 2048], Uc into uc [128,8], fc_b into [128,4].
  - Device (per core, feature-major layout: features on partitions,
    instances on the free dim):
      x^T   = relu(fc_W.T @ hT + b)            [512, n]   (PE fp32r + ACT)
      z^T   = W2.T @ x^T                       [2048, n]  (PE fp32r)
      g^T   = tanh(z_a) * sigmoid(z_b)         [1024, n]  (ACT + DVE)
      s     = uc.T @ g^T                       [1, n]     (PE fp32r)
      A     = sigmoid(s)                       [1, n]     -> output
      A_b   = ones.T @ A (partition broadcast) [128, n]   (PE fp32)
      Mp    = sum_f(x^T * A_b) over instances  [128, 4]   (DVE ttr) -> output
  - Host: M = sum(Mp)/sum(A); tiny VQ + classifier epilogue in fp32 numpy.
"""

import numpy as np

N_INST = 50000
D_IN, D_HID, D_ATT = 1024, 512, 256
N_ENS = 4
N_CLS = 2
CB_SIZE = 256
BETA = 0.25

N_CORES = 8
N_LOC = N_INST // N_CORES  # 6250
# 13 tiles, all >= 256 so fp32r matmuls run at 1 cycle/row.
# Small first tile = PE starts sooner; small last tile = short exposed tail.
TILE_SIZES = [500] * 11 + [494] + [256]
assert sum(TILE_SIZES) == N_LOC
NT_MAX = max(TILE_SIZES)
KA = D_IN // 128   # 8 k-chunks of h/fc_W
CH = D_HID // 128  # 4 chunks of hidden dim
JA = 2 * N_ENS * D_ATT // 128 // 2  # 8 (a,b) chunk pairs of the 2048 att dim

_CACHE = {}


def _build_nc():
    from contextlib import ExitStack

    import concourse.mybir as mybir
    import concourse.tile as tile
    from concourse import bacc

    F32 = mybir.dt.float32
    F32R = mybir.dt.float32r
    AF = mybir.ActivationFunctionType
    ALU = mybir.AluOpType

    nc = bacc.Bacc("TRN2", target_bir_lowering=False)

    hT = nc.dram_tensor("hT", [D_IN, N_LOC], F32R, kind="ExternalInput")
    fcW = nc.dram_tensor("fcW", [D_IN, D_HID], F32R, kind="ExternalInput")
    fcB = nc.dram_tensor("fcB", [128, CH], F32, kind="ExternalInput")
    W2 = nc.dram_tensor("W2", [D_HID, 2048], F32R, kind="ExternalInput")
    UC = nc.dram_tensor("UC", [128, KA], F32R, kind="ExternalInput")
    ONES = nc.dram_tensor("ONES", [1, 128], F32, kind="ExternalInput")
    A_out = nc.dram_tensor("A_out", [1, N_LOC], F32R, kind="ExternalOutput")
    M_out = nc.dram_tensor("M_out", [128, CH], F32, kind="ExternalOutput")

    with tile.TileContext(nc) as tc, ExitStack() as ctx:
        consts = ctx.enter_context(tc.tile_pool(name="consts", bufs=1))
        htp = ctx.enter_context(tc.tile_pool(name="htp", bufs=3))
        xp = ctx.enter_context(tc.tile_pool(name="xp", bufs=2))
        gp = ctx.enter_context(tc.tile_pool(name="gp", bufs=2))
        actp = ctx.enter_context(tc.tile_pool(name="actp", bufs=3))
        ap_pool = ctx.enter_context(tc.tile_pool(name="ap_pool", bufs=2))
        ps_x = ctx.enter_context(tc.tile_pool(name="ps_x", bufs=2, space="PSUM"))
        ps_z = ctx.enter_context(tc.tile_pool(name="ps_z", bufs=2, space="PSUM"))
        ps_s = ctx.enter_context(tc.tile_pool(name="ps_s", bufs=1, space="PSUM"))
        ps_a = ctx.enter_context(tc.tile_pool(name="ps_a", bufs=1, space="PSUM"))
        dramp = ctx.enter_context(tc.tile_pool(name="dramp", bufs=2, space="DRAM"))

        # First loads in exact need-order: fc weights for the first output
        # chunk, then the first h tile in two halves, so the PE starts after
        # ~0.75MB instead of the full 7MB of weights+tile.
        fcW_r = fcW.rearrange("(a p) m -> p a m", p=128)
        fc_sb = consts.tile([128, KA, D_HID], F32R)
        ht0 = htp.tile([128, KA, NT_MAX], F32R, name="ht")
        n0 = TILE_SIZES[0]
        hT0_r = hT[:, :n0].rearrange("(a p) n -> p a n", p=128)
        # need-ordered first loads: fc weights for output chunk 0, then the
        # first h tile in two halves
        nc.sync.dma_start(out=fc_sb[:, :, 0:128], in_=fcW_r[:, :, 0:128])
        nc.sync.dma_start(out=ht0[:, 0:2, :n0], in_=hT0_r[:, 0:2, :])
        nc.sync.dma_start(out=ht0[:, 2:KA, :n0], in_=hT0_r[:, 2:KA, :])
        for c in range(1, CH):
            nc.sync.dma_start(
                out=fc_sb[:, :, c * 128 : (c + 1) * 128],
                in_=fcW_r[:, :, c * 128 : (c + 1) * 128],
            )
        uc_sb = consts.tile([128, KA], F32R)
        nc.sync.dma_start(out=uc_sb, in_=UC[:, :])
        fcb_sb = consts.tile([128, CH], F32)
        nc.sync.dma_start(out=fcb_sb, in_=fcB[:, :])
        # W2 in (j, j+8) pair order so the first attention pairs land early
        W2_r = W2.rearrange("(b p) m -> p b m", p=128)
        w2_sb = consts.tile([128, CH, 2048], F32R)
        for j in range(JA):
            for jj in (j, j + 8):
                nc.sync.dma_start(
                    out=w2_sb[:, :, jj * 128 : (jj + 1) * 128],
                    in_=W2_r[:, :, jj * 128 : (jj + 1) * 128],
                )
        ones_sb = consts.tile([1, 128], F32)
        nc.sync.dma_start(out=ones_sb, in_=ONES[:, :])
        m_parts = consts.tile([128, CH, len(TILE_SIZES)], F32)

        off = 0
        for t, n_t in enumerate(TILE_SIZES):
            if t == 0:
                ht = ht0
            else:
                ht = htp.tile([128, KA, NT_MAX], F32R, name="ht")
                nc.sync.dma_start(
                    out=ht[:, :, :n_t],
                    in_=hT[:, off : off + n_t].rearrange("(a p) n -> p a n", p=128),
                )

            # fc + relu -> x^T in SBUF, [128, CH, n_t]
            x_sb = xp.tile([128, CH, NT_MAX], F32R, name="x_sb")
            for c in range(CH):
                x_ps = ps_x.tile([128, NT_MAX], F32, name="x_ps")
                for a in range(KA):
                    nc.tensor.matmul(
                        x_ps[:, :n_t],
                        lhsT=fc_sb[:, a, c * 128 : (c + 1) * 128],
                        rhs=ht[:, a, :n_t],
                        start=(a == 0),
                        stop=(a == KA - 1),
                    )
                nc.scalar.activation(
                    out=x_sb[:, c, :n_t],
                    in_=x_ps[:, :n_t],
                    func=AF.Relu,
                    bias=fcb_sb[:, c : c + 1],
                    scale=1.0,
                )

            # batch-ensemble attention: z pairs -> g = tanh(za)*sigmoid(zb)
            g_sb = gp.tile([128, JA, NT_MAX], F32R, name="g_sb")
            for j in range(JA):
                za_ps = ps_z.tile([128, NT_MAX], F32, name="za_ps")
                zb_ps = ps_z.tile([128, NT_MAX], F32, name="zb_ps")
                for b in range(CH):
                    nc.tensor.matmul(
                        za_ps[:, :n_t],
                        lhsT=w2_sb[:, b, j * 128 : (j + 1) * 128],
                        rhs=x_sb[:, b, :n_t],
                        start=(b == 0),
                        stop=(b == CH - 1),
                    )
                for b in range(CH):
                    nc.tensor.matmul(
                        zb_ps[:, :n_t],
                        lhsT=w2_sb[:, b, (j + 8) * 128 : (j + 9) * 128],
                        rhs=x_sb[:, b, :n_t],
                        start=(b == 0),
                        stop=(b == CH - 1),
                    )
                # tanh in place on the PSUM bank; sigmoid evacuates to SBUF
                # (DVE may read at most one PSUM operand)
                tb_sb = actp.tile([128, NT_MAX], F32, name="tb_sb")
                nc.scalar.activation(za_ps[:, :n_t], za_ps[:, :n_t], AF.Tanh)
                nc.scalar.activation(tb_sb[:, :n_t], zb_ps[:, :n_t], AF.Sigmoid)
                nc.vector.tensor_mul(
                    g_sb[:, j, :n_t], za_ps[:, :n_t], tb_sb[:, :n_t]
                )

            # attention scores s = uc . g, A = sigmoid(s)
            s_ps = ps_s.tile([1, NT_MAX], F32, name="s_ps")
            for a in range(KA):
                nc.tensor.matmul(
                    s_ps[:, :n_t],
                    lhsT=uc_sb[:, a : a + 1],
                    rhs=g_sb[:, a, :n_t],
                    start=(a == 0),
                    stop=(a == KA - 1),
                )
            a_sb = ap_pool.tile([1, NT_MAX], F32R, name="a_sb")
            nc.scalar.activation(a_sb[:, :n_t], s_ps[:, :n_t], AF.Sigmoid)
            nc.sync.dma_start(out=A_out[:, off : off + n_t], in_=a_sb[:, :n_t])

            # broadcast A to 128 partitions (exact fp32 outer product with ones)
            if t < len(TILE_SIZES) - 1:
                # broadcast via DRAM roundtrip: keeps the PE free mid-stream
                ab_sb = ap_pool.tile([128, NT_MAX], F32, name="ab_sb")
                a_dram = dramp.tile([1, NT_MAX], F32R, name="a_dram")
                nc.sync.dma_start(out=a_dram[:, :n_t], in_=a_sb[:, :n_t])
                nc.sync.dma_start(
                    out=ab_sb[:, :n_t],
                    in_=a_dram[0:1, :n_t].bitcast(F32).partition_broadcast(128),
                )
            else:
                # last tile: PE is draining anyway and the DMA roundtrip
                # latency would sit on the exposed tail - use the PE
                ab_ps = ps_a.tile([128, NT_MAX], F32, name="ab_ps")
                nc.tensor.matmul(
                    ab_ps[:, :n_t],
                    lhsT=ones_sb,
                    rhs=a_sb[:, :n_t].bitcast(F32),
                    start=True,
                    stop=True,
                )
                ab_sb = ab_ps  # mul may read one PSUM operand directly
            # weighted pooling: m_parts[:, c, t] = sum_n x^T[:,c,n] * A[n]
            wx_sb = ap_pool.tile([128, NT_MAX], F32, name="wx_sb")
            for c in range(CH):
                nc.vector.tensor_mul(
                    wx_sb[:, :n_t], x_sb[:, c, :n_t].bitcast(F32), ab_sb[:, :n_t]
                )
                nc.vector.tensor_reduce(
                    out=m_parts[:, c, t : t + 1],
                    in_=wx_sb[:, :n_t],
                    axis=mybir.AxisListType.X,
                    op=ALU.add,
                )
            off += n_t

        m_fin = consts.tile([128, CH], F32)
        for c in range(CH):
            nc.vector.tensor_reduce(
                out=m_fin[:, c : c + 1],
                in_=m_parts[:, c, :],
                axis=mybir.AxisListType.X,
                op=ALU.add,
            )
        nc.sync.dma_start(out=M_out[:, :], in_=m_fin)

    nc.compile()
    return nc


def _prep_shared(fc_W, fc_b, Ua, Ub, Uc):
    W2 = np.concatenate(
        [np.concatenate(list(Ua), axis=1), np.concatenate(list(Ub), axis=1)], axis=1
    ).astype(np.float32)  # [512, 2048]
    uc_cat = np.concatenate([Uc[e, :, 0] for e in range(N_ENS)])  # [1024]
    uc_host = np.ascontiguousarray(uc_cat.reshape(KA, 128).T).astype(np.float32)
    fcb_host = np.ascontiguousarray(fc_b.reshape(CH, 128).T).astype(np.float32)
    return W2, uc_host, fcb_host


def kernel(h, fc_W, fc_b, Ua, Ub, Uc, cls_W, cls_b, codebook):
    from concourse import bass_utils

    h = np.asarray(h, dtype=np.float32)
    if "nc" not in _CACHE:
        _CACHE["nc"] = _build_nc()
    nc = _CACHE["nc"]

    W2, uc_host, fcb_host = _prep_shared(
        np.asarray(fc_W, np.float32),
        np.asarray(fc_b, np.float32),
        np.asarray(Ua, np.float32),
        np.asarray(Ub, np.float32),
        np.asarray(Uc, np.float32),
    )
    hT = np.ascontiguousarray(h.T)  # [1024, 50000]
    fcW_h = np.ascontiguousarray(np.asarray(fc_W, np.float32))

    in_maps = []
    for c in range(N_CORES):
        in_maps.append(
            {
                "hT": np.ascontiguousarray(hT[:, c * N_LOC : (c + 1) * N_LOC]),
                "fcW": fcW_h,
                "fcB": fcb_host,
                "W2": W2,
                "UC": uc_host,
                "ONES": np.ones((1, 128), np.float32),
            }
        )

    import os
    import time

    trace = bool(int(os.environ.get("KERNEL_TRACE", "0")))
    t0 = time.time()
    res = bass_utils.run_bass_kernel_spmd(
        nc, in_maps, core_ids=list(range(N_CORES)), trace=trace
    )
    _CACHE["spmd_wall_s"] = time.time() - t0
    _CACHE["last_res"] = res
    results = res.results

    A = np.concatenate([r["A_out"] for r in results], axis=1).astype(np.float32)
    m_sum = np.zeros((128, CH), np.float32)
    for r in results:
        m_sum += r["M_out"]
    M_vec = m_sum.T.reshape(D_HID)  # hid index = c*128+p
    A_sum = np.float32(A.sum(dtype=np.float64).astype(np.float32))
    M = (M_vec / A_sum).astype(np.float32)[None, :]  # [1, 512]

    # --- tiny VQ + classifier epilogue (fp32, mirrors reference) ---
    codebook = np.asarray(codebook, np.float32)
    cls_W = np.asarray(cls_W, np.float32)
    cls_b = np.asarray(cls_b, np.float32)
    dist = (
        (M * M).sum(axis=1, keepdims=True)
        + (codebook * codebook).sum(axis=1)
        + 2.0 * (M @ codebook.T)
    )  # [1, CB]
    inds = np.argmin(dist, axis=1)
    q = codebook[inds]  # [1, 512]
    msq = ((q - M) ** 2).mean(dtype=np.float32)
    vq_loss = np.float32(msq * BETA + msq)

    logits = (M @ cls_W + cls_b).astype(np.float32)  # [1, 2]

    def _softmax(x):
        e = np.exp(x - x.max(axis=1, keepdims=True))
        return (e / e.sum(axis=1, keepdims=True)).astype(np.float32)

    y_probs = _softmax(logits)
    # top_k(y_probs[:, 1], 1) over a length-1 vector -> index 0
    top_idx = np.array([0])
    top_instance = logits[top_idx]
    Y_hat = np.argmax(top_instance, axis=1).astype(np.int32)[:, None]
    Y_prob = _softmax(top_instance)

    return (
        top_instance,
        Y_prob,
        Y_hat,
        np.float32(vq_loss),
        y_probs,
        A,
    )
